# revision 46
# baseline (speedup 1.0000x reference)
"""Trainium2 Bass kernel for DKNN soft top-k (NeuralSort, deterministic).

Reference computation (per query row q, N=1024 neighbors, D=512, K=16):
    scores[n]  = -||q - nb[n]||^2  =  2 q.nb[n] - ||nb[n]||^2 - ||q||^2
    B[n]       = sum_m |scores[n] - scores[m]|
    logits[k,n] = (coef[k]*scores[n] - B[n]) / TAU,  coef[k] = N+1-2(k+1)
    P_hat      = softmax_n(logits)
    out[n]     = sum_k P_hat[k, n]

Sharding: data-parallel over Q (64 queries -> 8 cores x 8 queries).
Each core holds the full neighbor bank.

Per-core layout highlights:
  - scores s8 [8(q), 1024(n)] computed on PE from transposed operands,
    with -||n||^2 folded in as a K=1 matmul row and -||q||^2 added at evac.
  - B via one fused pass per (q, n-tile): |bcast_q - s_col| + row-sum in a
    single instruction (ACT: activation(Abs, bias, accum_out); DVE:
    scalar_tensor_tensor(add, abs_max vs zeros, accum_out)).
  - bcast_q [128, 1024] (score row replicated across partitions) built by
    GPSIMD partition_broadcast (or PE selector matmul fallback).
  - softmax for all (q, k) pairs at once: partitions = 8q x 16k = 128,
    logits built by PE matmuls with coef/minus-one selector weights, exp+sum
    on ACT, final sum_k P_hat via PE matmul with (1/Z)-scaled selector.
"""

import os
from contextlib import ExitStack

import numpy as np

import concourse.bass as bass
import concourse.tile as tile
from concourse import library_config, mybir
from concourse.bass_utils import run_bass_kernel_spmd

F32 = mybir.dt.float32

Q, N, D, K = 64, 1024, 512, 16
TAU = 1.0
NCORES = 8
QS = Q // NCORES  # 8 queries per core
NT = N // 128     # 8 n-tiles
DT = D // 128     # 4 d-tiles
COEF = (N + 1 - 2 * (np.arange(K) + 1)).astype(np.float32)  # [K] = 1023, 1021, ...

# How the B-phase (q, t) instruction stream is split across engines.
# Pattern entries: 'A' -> ScalarE(ACT), 'V' -> VectorE(DVE), 'G' -> GPSIMD.
B_PATTERN = os.environ.get("DKNN_B_PATTERN", "AV")
BCAST_MODE = os.environ.get("DKNN_BCAST", "dma")  # dma | pe | gpsimd
SKIP_B = os.environ.get("DKNN_SKIP_B", "0") == "1"
SKIP_TAIL = os.environ.get("DKNN_SKIP_TAIL", "0") == "1"
Q0_GPSIMD = os.environ.get("DKNN_Q0_GPSIMD", "0") == "1"
SKIP_BCAST = os.environ.get("DKNN_SKIP_BCAST", "0") == "1"


def _consts() -> dict[str, np.ndarray]:
    ident = np.eye(128, dtype=np.float32)
    sel16c = np.zeros((8, 128), np.float32)   # coef-weighted (q,k) selector
    nsel16 = np.zeros((8, 128), np.float32)   # -1 (q,k) selector
    sel16t = np.zeros((128, 8), np.float32)   # transposed 0/1 selector
    for p in range(128):
        qq, kk = divmod(p, 16)
        sel16c[qq, p] = COEF[kk] / TAU
        nsel16[qq, p] = -1.0 / TAU
        sel16t[p, qq] = 1.0
    sel_all = np.zeros((8, 8 * 128), np.float32)  # per-q broadcast selector
    for q in range(8):
        sel_all[q, 128 * q:128 * (q + 1)] = 1.0
    c8 = np.concatenate([sel_all, sel16c, nsel16], axis=1)      # [8, 1280]
    c128 = np.concatenate([ident, sel16t], axis=1)              # [128, 136]
    return dict(c8=c8, c128=c128)


def build_nc(replicate: int = 1) -> bass.Bass:
    """Build the single-core Bass program (same NEFF runs SPMD on all cores).

    replicate > 1 repeats the whole compute body (for wall-clock timing
    amplification in benchmarks); outputs are just overwritten each round.
    """
    nc = bass.Bass("TRN2", target_bir_lowering=False, debug=False)
    nc._dknn_s8_dram = nc.dram_tensor("s8_stash", [QS, N], F32)

    q_d = nc.dram_tensor("query", [QS, D], F32, kind="ExternalInput")
    nb_d = nc.dram_tensor("neighbors", [N, D], F32, kind="ExternalInput")
    c8_d = nc.dram_tensor("c8", [8, 1280], F32, kind="ExternalInput")
    c128_d = nc.dram_tensor("c128", [128, 136], F32, kind="ExternalInput")
    out_d = nc.dram_tensor("out", [QS, N], F32, kind="ExternalOutput")

    with tile.TileContext(nc) as tc, ExitStack() as ctx:
        pin = ctx.enter_context(tc.tile_pool(name="pin", bufs=1))
        pwork = ctx.enter_context(tc.tile_pool(name="pwork", bufs=1))
        pbc = ctx.enter_context(tc.tile_pool(name="pbc", bufs=8))

        # ---- persistent consts / inputs (packed: 2 DMAs)
        c8 = pin.tile([8, 1280], F32)
        nc.sync.dma_start(c8[:], c8_d.ap())
        c128 = pin.tile([128, 136], F32)
        nc.scalar.dma_start(c128[:], c128_d.ap())
        sel_all = c8[:, 0:1024]
        sel16c = c8[:, 1024:1152]
        nsel16 = c8[:, 1152:1280]
        ones18 = c8[0:1, 0:8]       # sel_all row 0 starts with ones
        ident = c128[:, 0:128]
        sel16t = c128[:, 128:136]

        zeros = pin.tile([128, N], F32)
        nc.vector.memset(zeros[:], 0.0)

        nc._dknn_c8 = c8
        nc._dknn_c128 = c128
        nc._dknn_ps_warmp = ctx.enter_context(
            tc.tile_pool(name="ps_warmp", bufs=1, space="PSUM"))

        if BCAST_MODE == "gpsimd" or (BCAST_MODE == "dma" and Q0_GPSIMD):
            nc.gpsimd.load_library(library_config.proxy)

        for rep in range(replicate):
            _emit_body(ctx, tc, rep, pwork, pbc,
                       q_d, nb_d, out_d,
                       ident, sel16c, nsel16, sel16t, ones18, sel_all, zeros)


    _legalize_single_wait(nc)
    return nc


_LEGALIZE_FAILURES: list = []
_NOP_UID = 0


def _legalize_single_wait(nc):
    """Split multi-wait instructions: one wait stays, others move to NoOps
    inserted directly before (same engine). The sequencer processes waits
    in stream order either way, so semantics are identical; the pinned
    walrus codegen just requires <= 1 sync-wait per instruction.
    """
    global _NOP_UID
    import bass_rust
    from bass_rust import SyncInfo

    _LEGALIZE_FAILURES.clear()
    for fn in nc.m.functions:
        for blk in fn.blocks:
            il = blk.instructions
            if not any(ins.has_wait for ins in il):
                continue
            i = -1
            while i + 1 < len(il):
                i += 1
                ins = il[i]
                if type(ins).__name__ == "InstDrain":
                    # drains encode multi-waits through their own path...
                    # actually they don't: split them too
                    pass
                si = getattr(ins, "sync_info", None)
                waits = list(si.on_wait) if si is not None and si.on_wait else []
                if len(waits) <= 1:
                    continue
                for w in waits[:-1]:
                    _NOP_UID += 1
                    nop = bass_rust.InstNoOp(
                        name=f"LEGNOP-{_NOP_UID}",
                        engine=ins.engine, ins=[], outs=[],
                        sync_info=SyncInfo(on_wait=[w], on_update=[]))
                    il.insert(i, nop)
                    i += 1
                ins.sync_info = SyncInfo(on_wait=waits[-1:],
                                         on_update=list(si.on_update))


def _emit_body(ctx, tc, rep, pwork, pbc, q_d, nb_d, out_d,
               ident, sel16c, nsel16, sel16t, ones18, sel_all, zeros):
    nc = tc.nc

    # fresh tiles each replication round (same tags -> reuse slots across reps)
    def wtile(shape, tag):
        return pwork.tile(shape, F32, name=f"{tag}_r{rep}", tag=tag)

    q_nat = wtile([QS, D], "q_nat")
    nc.sync.dma_start(q_nat[:], q_d.ap())
    nb_big = wtile([128, NT * D], "nb_big")
    for g in range(4):
        eng = nc.sync if g % 2 == 0 else nc.scalar
        eng.dma_start(
            nb_big[:, 2 * D * g:2 * D * (g + 1)].rearrange(
                "p (t d) -> p t d", t=2),
            nb_d.ap()[256 * g:256 * (g + 1), :].rearrange(
                "(t p) d -> p t d", p=128))
    nb = [nb_big[:, D * t:D * (t + 1)] for t in range(NT)]

    sq_scr_a = wtile([128, D], "sq_scr_a")   # ACT square scratch
    sq_scr_v = wtile([128, D], "sq_scr_v")   # DVE square scratch
    nsq_colT = wtile([128, NT], "nsq_colT")
    qsq = wtile([QS, 1], "qsq")
    negqsq = wtile([QS, 1], "negqsq")
    qT2 = wtile([128, DT * QS], "qT2")
    negnsqT8 = wtile([NT, 128], "negnsqT8")
    s8 = wtile([QS, N], "s8")
    negscolT = wtile([128, NT * QS], "negscolT")
    BcolT = wtile([128, NT * QS], "BcolT")
    B8 = wtile([QS, N], "B8")
    scrA = wtile([128, N], "scrA")
    scrD = wtile([128, N], "scrD")
    negmax0 = wtile([128, 1], "negmax0")
    negmax1 = wtile([128, 1], "negmax1")
    negmax = wtile([128, 1], "negmax")
    E = wtile([128, N], "E")
    Zs = wtile([128, 1], "Zs")
    Rz = wtile([128, 1], "Rz")
    rzsel = wtile([128, 8], "rzsel")
    out_sb = wtile([QS, N], "out_sb")

    # ---- PE warm-up: dummy matmuls while input DMAs are in flight.
    # They also pre-sync the PE on the const-DMA semaphore lanes so later
    # matmuls reading ident/sel* need no extra wait (LW allows only one).
    c8 = nc._dknn_c8
    c128 = nc._dknn_c128
    if rep == 0:
        ps_warm = nc._dknn_ps_warmp.tile([8, 64], F32, name="ps_warm",
                                         tag="ps_warm")
        nc._dknn_ps_warm = ps_warm
        idb = ident.bitcast(mybir.dt.bfloat16)
        c8b = c8.bitcast(mybir.dt.bfloat16)
        for i in range(24):
            if i % 2 == 0:
                nc.tensor.matmul(ps_warm[:], idb[:, 0:8], idb[:, 0:64],
                                 start=True, stop=True)
            else:
                nc.tensor.matmul(ps_warm[:, 0:64], c8b[:, 0:8], c8b[:, 0:64],
                                 start=True, stop=True)
    ps_warm = nc._dknn_ps_warm

    def sponge(ap):
        # tiny bf16 matmul reading `ap` (bitcast) - absorbs multi-sem waits
        # so the following fp32 matmul/transpose needs at most one wait.
        b = ap.bitcast(mybir.dt.bfloat16)
        nc.tensor.matmul(ps_warm[0:b.shape[0] if b.shape[0] < 8 else 8, 0:8],
                         b[:, 0:8], b[:, 0:8], start=True, stop=True)

    # ---- head: neighbor norms (split ACT/DVE) and transposes
    # emitted per DMA chunk (t pairs) so work starts as data lands
    nT_all = wtile([128, DT * N], "nT_all")   # layout: col 1024*d + n

    with tc.tile_pool(name=f"ps_head{rep}", bufs=2, space="PSUM") as ps_head, \
         tc.tile_pool(name=f"ps_small{rep}", bufs=1, space="PSUM") as ps_small, \
         tc.tile_pool(name=f"ps_s8{rep}", bufs=1, space="PSUM") as ps_s8p:

        ps_sm = ps_small.tile([128, 128], F32, name="ps_sm", tag="ps_sm")

        for t in range(NT):
            if t % 2 == 0:
                sponge(nb_big[:, D * t:D * t + 8])
            if t >= 2:
                # absorb the ps_t slot-recycle WAR (DVE evac of tile t-2)
                sponge(nT_all[:, 128 * (t - 2):128 * (t - 2) + 8])
            # norms on ACT; nT evacs all on DVE so the s8 matmuls wait on
            # a single semaphore (matmul LW slots encode very few waits)
            nc.scalar.activation(sq_scr_a[:], nb[t],
                                 mybir.ActivationFunctionType.Square,
                                 accum_out=nsq_colT[:, t:t + 1])
            # transposes of this n-tile for all 4 d-tiles
            ps_t = ps_head.tile([128, DT * 128], F32, name="ps_t", tag="ps_t")
            for d in range(DT):
                nc.tensor.transpose(ps_t[:, 128 * d:128 * (d + 1)],
                                    nb[t][:, 128 * d:128 * (d + 1)],
                                    ident)
            dst = nT_all[:].rearrange("p (d n) -> p d n", d=DT)[:, :,
                                                               128 * t:128 * (t + 1)]
            nc.vector.tensor_copy(dst, ps_t[:].rearrange(
                "p (d j) -> p d j", d=DT))

        # query norm and transpose
        nc.scalar.activation(sq_scr_a[:QS, :], q_nat[:],
                             mybir.ActivationFunctionType.Square,
                             accum_out=qsq[:])
        nc.scalar.activation(negqsq[:], qsq[:],
                             mybir.ActivationFunctionType.Copy, scale=-1.0)
        for d in range(DT):
            nc.tensor.transpose(ps_sm[:, QS * d:QS * (d + 1)],
                                q_nat[:, 128 * d:128 * (d + 1)],
                                ident[0:QS, 0:QS])
        nc.vector.tensor_scalar_mul(qT2[:], ps_sm[:, 0:DT * QS], 2.0)

        # nsq row: [128, 8] -> [8, 128], negated at evac, then flattened to
        # [1, 1024] by a partition-major SBUF->SBUF DMA (row j ++ row j+1 ..)
        sponge(nsq_colT[:, 0:4])
        nc.tensor.transpose(ps_sm[0:NT, :], nsq_colT[:], ident)
        nc.vector.tensor_scalar_mul(negnsqT8[:], ps_sm[0:NT, :], -1.0)
        negnsq_row = wtile([1, N], "negnsq_row")
        nc.scalar.dma_start(negnsq_row[:], negnsqT8[:])

        # scores s8 = 2 q.nT - ||n||^2 (PE), - ||q||^2 folded in at evac
        ps8 = ps_s8p.tile([QS, N], F32, name="ps8", tag="ps8")
        for h in range(2):
            for d in range(DT):
                nc.tensor.matmul(ps8[:, 512 * h:512 * (h + 1)],
                                 qT2[:, QS * d:QS * (d + 1)],
                                 nT_all[:, N * d + 512 * h:N * d + 512 * (h + 1)],
                                 start=(d == 0), stop=False)
            nc.tensor.matmul(ps8[:, 512 * h:512 * (h + 1)],
                             ones18, negnsq_row[:, 512 * h:512 * (h + 1)],
                             start=False, stop=True)
        nc.scalar.activation(s8[:], ps8[:],
                             mybir.ActivationFunctionType.Identity,
                             bias=negqsq[:])

        # negated score columns: -s'[q, n] laid out [128(n%128), 8t+q]
        sponge(s8[:, 0:8])
        for t in range(NT):
            nc.tensor.transpose(ps_sm[:, QS * t:QS * (t + 1)],
                                s8[:, 128 * t:128 * (t + 1)],
                                ident[0:QS, 0:QS])
        nc.vector.tensor_scalar_mul(negscolT[:], ps_sm[:, 0:NT * QS], -1.0)

        if BCAST_MODE == "dma":
            # wait-free ACT landing slot for the stash DMA's hoisted wait
            nc.scalar.activation(sq_scr_a[0:1, 0:1], s8[0:1, 0:1],
                                 mybir.ActivationFunctionType.Copy)
            nc.scalar.dma_start(nc._dknn_s8_dram.ap(), s8[:])

    # ---- B phase (t-outer) + interleaved tail halves ----
    with tc.tile_pool(name=f"ps_bt{rep}", bufs=1, space="PSUM") as ps_bt:
        ps_b8 = ps_bt.tile([QS, N], F32, name="ps_b8", tag="ps_b8")
        ps_log = ps_bt.tile([128, N], F32, name="ps_log", tag="ps_log")
        ps_out = ps_bt.tile([QS, N], F32, name="ps_out", tag="ps_out")

        # engine schedule: ACT does a fused |x|+rowsum instruction; DVE
        # needs two passes (diff, then abs-reduce), so give ACT ~2x tiles.
        n_act = int(os.environ.get("DKNN_B_NACT", "40"))
        sched = []
        acc = 0
        for i in range(QS * NT):
            acc += n_act
            if acc >= QS * NT:
                acc -= QS * NT
                sched.append("A")
            else:
                sched.append("V")

        def b_instr(bcq, t, q):
            c = QS * t + q
            col = negscolT[:, c:c + 1]
            if sched[q * NT + t] == "A":
                nc.scalar.activation(scrA[:], bcq[:],
                                     mybir.ActivationFunctionType.Abs,
                                     bias=col, accum_out=BcolT[:, c:c + 1])
            else:
                nc.vector.tensor_scalar(scrD[:], bcq[:], col, None,
                                        op0=mybir.AluOpType.add)
                nc.vector.tensor_reduce(BcolT[:, c:c + 1], scrD[:],
                                        axis=mybir.AxisListType.X,
                                        op=mybir.AluOpType.add,
                                        apply_absolute_value=True)

        def half_tail(h):
            # B8 evac + logits for n in [512h, 512h+512), after groups 4h..4h+3
            nc.vector.tensor_copy(B8[:, 512 * h:512 * (h + 1)],
                                  ps_b8[:, 512 * h:512 * (h + 1)])
            nc.tensor.matmul(ps_log[:, 512 * h:512 * (h + 1)],
                             sel16c, s8[:, 512 * h:512 * (h + 1)],
                             start=True, stop=False)
            nc.tensor.matmul(ps_log[:, 512 * h:512 * (h + 1)],
                             nsel16, B8[:, 512 * h:512 * (h + 1)],
                             start=False, stop=True)
            nm = negmax0 if h == 0 else negmax1
            nc.vector.tensor_reduce(nm[:], ps_log[:, 512 * h:512 * (h + 1)],
                                    axis=mybir.AxisListType.X,
                                    op=mybir.AluOpType.max, negate=True)

        # pre-sync ACT on negscolT (DVE) so B instrs wait only on their DMA
        nc.scalar.activation(rzsel[:, 0:1], negscolT[:, 0:1],
                             mybir.ActivationFunctionType.Copy)
        for q in range(QS):
            bcq = pbc.tile([128, N], F32, name="bcq", tag="bcq")
            if SKIP_BCAST:
                nc.vector.memset(bcq[:1, :1], 0.0)
            elif BCAST_MODE == "dma":
                if q == 0 and Q0_GPSIMD:
                    nc.gpsimd.partition_broadcast(bcq[:], s8[0:1, :])
                else:
                    # ACT ring + a wait-free ACT sponge per DMA: hoisted
                    # waits land on compute, which SP's DMA-only stream lacks
                    nc.scalar.activation(sq_scr_a[0:1, 0:1], s8[0:1, 0:1],
                                         mybir.ActivationFunctionType.Copy)
                    nc.scalar.dma_start(
                        bcq[:],
                        nc._dknn_s8_dram.ap()[q:q + 1, :].broadcast_to([128, N]))
            else:
                nc.gpsimd.partition_broadcast(bcq[:], s8[q:q + 1, :])
            # keep PE p-state alive through the phase (bf16: multi-wait ok)
            bcb = bcq.bitcast(mybir.dt.bfloat16)
            nc.tensor.matmul(ps_warm[:], bcb[:, 0:8], bcb[:, 0:64],
                             start=True, stop=True)
            for t in range(NT):
                if SKIP_B:
                    nc.vector.memset(BcolT[:, QS * t + q:QS * t + q + 1], 1.0)
                else:
                    b_instr(bcq, t, q)
                if q == QS - 1:
                    # tile t complete: transpose its B columns into ps_b8
                    sponge(BcolT[:, QS * t:QS * t + 4])
                    nc.tensor.transpose(ps_b8[:, 128 * t:128 * (t + 1)],
                                        BcolT[:, QS * t:QS * (t + 1)],
                                        ident)
                    if t == 3:
                        half_tail(0)
                    if t == 5:
                        # ramp the PE back up before the tail matmuls
                        for _ in range(8):
                            nc.tensor.matmul(ps_warm[:],
                                             BcolT[:, QS * t:QS * t + 8],
                                             ident[:, 0:64],
                                             start=True, stop=True)
        half_tail(1)

        if SKIP_TAIL:
            nc.vector.tensor_copy(out_sb[:], s8[:])
            nc.sync.dma_start(out_d.ap(), out_sb[:])
            return

        # combine halves: negmax = min(negmax0, negmax1) = -max(max0, max1)
        nc.vector.tensor_tensor(negmax[:], negmax0[:], negmax1[:],
                                op=mybir.AluOpType.min)
        nc.scalar.activation(E[:], ps_log[:],
                             mybir.ActivationFunctionType.Exp,
                             bias=negmax[:], accum_out=Zs[:])
        nc.vector.reciprocal(Rz[:], Zs[:])
        nc.scalar.activation(rzsel[:], sel16t,
                             mybir.ActivationFunctionType.Copy,
                             scale=Rz[:])

        for h in range(2):
            nc.tensor.matmul(ps_out[:, 512 * h:512 * (h + 1)],
                             rzsel[:], E[:, 512 * h:512 * (h + 1)],
                             start=True, stop=True)
        nc.scalar.copy(out_sb[:], ps_out[:])
        nc.scalar.dma_start(out_d.ap(), out_sb[:])


_CACHE: dict = {}


def _get_nc() -> bass.Bass:
    if "nc" not in _CACHE:
        _CACHE["nc"] = build_nc()
    return _CACHE["nc"]


def kernel(query: np.ndarray, neighbors: np.ndarray) -> np.ndarray:
    query = np.ascontiguousarray(query, dtype=np.float32)
    neighbors = np.ascontiguousarray(neighbors, dtype=np.float32)
    assert query.shape == (Q, D) and neighbors.shape == (N, D)

    nc = _get_nc()
    consts = _consts()
    in_maps = []
    for c in range(NCORES):
        m = dict(consts)
        m["query"] = query[c * QS:(c + 1) * QS]
        m["neighbors"] = neighbors
        in_maps.append(m)

    res = run_bass_kernel_spmd(nc, in_maps, core_ids=list(range(NCORES)))
    out = np.concatenate([res.results[c]["out"] for c in range(NCORES)], axis=0)
    return out.astype(np.float32)
